# revision 32
# baseline (speedup 1.0000x reference)
"""Trainium2 Bass kernel for the GCA sparse-attention module.

Math (per batch b):
    a  = emb_a[word_seq] @ lin_w + lin_b                    # [W, H]
    u  = hidden @ a.T / sqrt(H)                             # [L, W]
    e  = exp(u) * (label > 0)                               # [L, W]
    p  = e / (sum_w e + 1e-10)
    o  = sum_w p * emb_c[label]                             # [L, H]

Structure:
  * u = hidden @ (g @ lin_w + lb).T = (hidden @ lin_w.T_ext) @ g_ext.T
    with lin_b folded in via an extra 1.0 column in the padded gathered
    rows; the embedding lookup + transpose is host-side packing so the
    device sees plain dense DMAs (no indirect gather, no PE transposes).
  * the H-contraction (hidden @ lwx) runs in fp8e4 DoubleRow (2 rows of
    weights per PE cell -> 9 matmuls instead of 18, half the DMA bytes).
    lin_w is pre-scaled by 16 to stay clear of fp8 subnormals; the 1/16
    is absorbed into the exp() input scale.  u's accuracy needs are tiny
    (exp arguments are O(0.01)), so fp8's ~6% error is harmless here.
  * u is computed TRANSPOSED (uT[w, l]) so the per-label masked sums
    qe[n, l] = sum_w e*[label==n] reduce over the PARTITION axis — done
    on the PE as 10 accumulating matmuls with constant one-hot lhsT
    weights.  Each one-hot column ALSO carries a 1 in column 0, so PSUM
    row 0 accumulates the normalizer s[l] = sum_w e*[label>0] for free.
  * s is transposed to the partition axis by two tiny PE matmuls;
    r = 1/(s+eps) runs on [128,1] tiles (1 elem/lane — fast) and is
    applied as a per-partition scale during the PSUM->SBUF output
    copies.  o = qeT.T @ emb_c needs no transpose.
  * DMAs: each HWDGE dma_start costs ~0.6-0.8us of sequencer issue
    time, so inputs ship as a few large transfers per ring, ordered by
    when compute needs them.

Sharding: 8 cores = (batch b, L-half) pairs; fully independent.
"""

import numpy as np
import ml_dtypes

import concourse.bass as bass
import concourse.mybir as mybir
import concourse.tile as tile
from concourse import bacc
from concourse import bass_utils

# Problem shapes (hardcoded per contract).
B, L, W = 4, 512, 256
VOCAB, E, H = 30000, 300, 768
EP = 384                    # E padded: 300 data + 83 zero + 1 ones/bias col
NL = 6
P = 128
NCORES = 8
LC = L * B // NCORES        # 256 l-rows per core
WT = W // P                 # 2 w-tiles
LT = LC // P                # 2 l-tiles
HT = H // P                 # 6 h-tiles
ET = EP // P                # 3 e-chunks
TEMPER = float(H) ** 0.5
LWS = 16.0                  # lin_w fp8 pre-scale
GS = 16.0                   # gathered-rows fp8 pre-scale
OHC = 48                    # one-hot constant block

F32 = mybir.dt.float32
BF16 = mybir.dt.bfloat16
F8 = mybir.dt.float8e4
I8 = mybir.dt.int8
BF = ml_dtypes.bfloat16
F8NP = ml_dtypes.float8_e4m3fn

TRACE = False  # test.py flips this for profiled runs

_CACHE = {}


def _build():
    """Build + compile the per-core Bass program (identical on all cores)."""
    nc = bacc.Bacc("TRN2", debug=False, num_devices=1)

    # hT packed on host: hT[p, m*LC + l] = hidden[l, m*128 + p]  (fp8)
    hT_d = nc.dram_tensor("hT", [P, HT * LC], F8, kind="ExternalInput").ap()
    # lwx packed on host: lwx[p, m*EP + e] = 16 * lin_w.T_ext[m*128+p, e]
    lwx_d = nc.dram_tensor("lwx", [P, HT * EP], F8, kind="ExternalInput").ap()
    # gathered+extended+transposed rows: gT[p, c*W + w] = 16 * g_ext[w, c*128+p]
    gT_d = nc.dram_tensor("gT", [P, ET * W], F8, kind="ExternalInput").ap()
    # labels TRANSPOSED+packed: lab[p, j*LC + l] = label[l, j*128 + p]
    lab_d = nc.dram_tensor("label", [P, WT * LC], I8, kind="ExternalInput").ap()
    # emb_c rows 1..5 in rows 1..5 of an 8-row tensor (rows 0,6,7 zero)
    ec_d = nc.dram_tensor("emb_c", [8, H], BF16, kind="ExternalInput").ap()
    # one-hots: col 8n+(n+1) = label-(n+1) hot, col 8n+0 = 1 (normalizer
    # -> pqe row 0); row 0 of cols 40..47 = ones (s-transpose rhs)
    oh_d = nc.dram_tensor("oh", [P, OHC], BF16, kind="ExternalInput").ap()
    o_d = nc.dram_tensor("o", [LC, H], BF16, kind="ExternalOutput").ap()

    # split 4+2 h-chunks so each DoubleRow pair lives in one tile
    HALF_E = 4 * EP
    HALF_H = 4 * LC

    with tile.TileContext(nc) as tc:
        with (
            tc.tile_pool(name="cst", bufs=1) as cst,
            tc.tile_pool(name="sb", bufs=1) as sb,
            tc.tile_pool(name="wrk", bufs=4) as wrk,
            tc.tile_pool(name="psA", bufs=3, space="PSUM") as psA,
            tc.tile_pool(name="psO", bufs=3, space="PSUM") as psO,
            tc.tile_pool(name="psR", bufs=1, space="PSUM") as psR,
        ):
            # ---- PE clock warmup: HAM un-throttles 1.2->2.4 GHz after
            # ~3.4us of sustained matmul activity; burn the DMA wait on
            # dummy matmuls so the real ones run warm ----
            warm = cst.tile([P, 4 * P], BF16, name="warm")
            nc.gpsimd.memset(warm[:], 0.0)
            pwarm = psR.tile([P, 4 * P], F32, name="pwarm", tag="psr")
            for _ in range(5):
                nc.tensor.matmul(
                    out=pwarm[:], lhsT=warm[:, :P], rhs=warm[:],
                    start=True, stop=True,
                )
            # ---- input DMAs: few big transfers, ordered by need-time ----
            # scalar HWDGE ring
            lwxA = sb.tile([P, HALF_E], F8, name="lwxA", tag="lwxA")
            nc.scalar.dma_start(out=lwxA[:], in_=lwx_d[:, :HALF_E])
            lwxB = sb.tile([P, HT * EP - HALF_E], F8, name="lwxB", tag="lwxB")
            nc.scalar.dma_start(out=lwxB[:], in_=lwx_d[:, HALF_E:])
            oh = cst.tile([P, OHC], BF16, name="oh")
            nc.scalar.dma_start(out=oh[:], in_=oh_d)
            ec = cst.tile([8, H], BF16, name="ec")
            nc.scalar.dma_start(out=ec[:], in_=ec_d)
            # sync HWDGE ring
            hTA = sb.tile([P, HALF_H], F8, name="hTA", tag="hTA")
            nc.sync.dma_start(out=hTA[:], in_=hT_d[:, :HALF_H])
            hTB = sb.tile([P, HT * LC - HALF_H], F8, name="hTB", tag="hTB")
            nc.sync.dma_start(out=hTB[:], in_=hT_d[:, HALF_H:])
            gTt = sb.tile([P, ET * W], F8, name="gTt", tag="gTt")
            nc.sync.dma_start(out=gTt[:], in_=gT_d)
            lab = cst.tile([P, WT * LC], I8, name="lab")
            nc.sync.dma_start(out=lab[:], in_=lab_d)

            # 3D views for DoubleRow k-subtile pairs
            lwxA3 = lwxA[:].rearrange("p (m e) -> p m e", m=4)
            lwxB3 = lwxB[:].rearrange("p (m e) -> p m e", m=2)
            hTA3 = hTA[:].rearrange("p (m l) -> p m l", m=4)
            hTB3 = hTB[:].rearrange("p (m l) -> p m l", m=2)

            # ---- label masks (DVE, overlaps h2t) ----
            labf = sb.tile([P, WT * LC], BF16, name="labf", tag="labf")
            nc.vector.tensor_copy(out=labf[:], in_=lab[:])
            masks = []
            for n in range(1, NL):
                t = sb.tile([P, WT * LC], BF16, name=f"msk{n}", tag=f"msk{n}")
                nc.vector.tensor_scalar(
                    out=t[:], in0=labf[:],
                    scalar1=float(n), scalar2=None,
                    op0=mybir.AluOpType.is_equal,
                )
                masks.append(t)

            # ---- h2xT[c] = (hidden @ lwx)ᵀ chunk: fp8 DoubleRow ----
            pe = [psA.tile([P, LC], F32, name=f"pe{c}", tag="acc") for c in range(ET)]
            for mp in range(HT // 2):       # pair index: h-chunks (2mp, 2mp+1)
                lw3, h3, mm = (lwxA3, hTA3, 2 * mp) if mp < 2 else (lwxB3, hTB3, 0)
                for c in range(ET):
                    nc.tensor.matmul(
                        out=pe[c][:],
                        lhsT=lw3[:, mm : mm + 2, c * P : (c + 1) * P],
                        rhs=h3[:, mm : mm + 2, :],
                        start=(mp == 0),
                        stop=(mp == HT // 2 - 1),
                        perf_mode=mybir.MatmulPerfMode.DoubleRow,
                    )
            h2tt = sb.tile([P, ET * LC], F8, name="h2tt", tag="h2tt")
            for c in range(ET):
                sl = h2tt[:, c * LC : (c + 1) * LC]
                if c == 1:
                    nc.scalar.copy(out=sl, in_=pe[c][:])
                else:
                    nc.vector.tensor_copy(out=sl, in_=pe[c][:])

            # ---- uT per w-tile + exp: eT[:, jLC:] = exp(uT/temper) ----
            # fp8 DoubleRow over e-chunk pair (0,1), plain fp8 MM for chunk 2
            gT3 = gTt[:].rearrange("p (c w) -> p c w", c=3)
            h2t3 = h2tt[:].rearrange("p (c l) -> p c l", c=3)
            pu = [psA.tile([P, LC], F32, name=f"pu{j}", tag="acc") for j in range(WT)]
            for j in range(WT):
                nc.tensor.matmul(
                    out=pu[j][:],
                    lhsT=gT3[:, 0:2, j * P : (j + 1) * P],
                    rhs=h2t3[:, 0:2, :],
                    start=True, stop=False,
                    perf_mode=mybir.MatmulPerfMode.DoubleRow,
                )
                nc.tensor.matmul(
                    out=pu[j][:],
                    lhsT=gT3[:, 2:3, j * P : (j + 1) * P],
                    rhs=h2t3[:, 2:3, :],
                    start=False, stop=True,
                )
            eT = sb.tile([P, WT * LC], BF16, name="eT", tag="eT")
            for j in range(WT):
                nc.scalar.activation(
                    out=eT[:, j * LC : (j + 1) * LC], in_=pu[j][:],
                    func=mybir.ActivationFunctionType.Exp,
                    scale=1.0 / (LWS * GS * TEMPER),
                )

            # ---- qeT[n, l] rows 1..5 + normalizer row 0, all on the PE ----
            pqe = psA.tile([8, LC], F32, name="pqe", tag="acc")
            k = 0
            for n in range(1, NL):
                scr = wrk.tile([P, WT * LC], BF16, name="scr", tag="scr")
                nc.vector.tensor_mul(out=scr[:], in0=masks[n - 1][:], in1=eT[:])
                for j in range(WT):
                    nc.tensor.matmul(
                        out=pqe[:],
                        lhsT=oh[:, (n - 1) * 8 : n * 8],
                        rhs=scr[:, j * LC : (j + 1) * LC],
                        start=(k == 0),
                        stop=(k == (NL - 1) * WT - 1),
                    )
                    k += 1

            # ---- r[l] = 1/(s+eps): transpose s to partitions, then scale
            srow = sb.tile([1, LC], BF16, name="srow", tag="srow")
            nc.vector.tensor_copy(out=srow[:], in_=pqe[0:1, :])
            rr = []
            for i in range(LT):
                # i=1 reuses the long-dead warmup bank; i=0 gets its own
                psr = psR.tile([P, 1], F32, name=f"psr{i}",
                               tag="psr" if i else "psr0")
                nc.tensor.matmul(
                    out=psr[:],
                    lhsT=srow[:, i * P : (i + 1) * P],
                    rhs=oh[0:1, 40:41],
                    start=True,
                    stop=True,
                )
                r = sb.tile([P, 1], F32, name=f"r{i}", tag=f"r{i}")
                nc.vector.tensor_scalar_add(out=r[:], in0=psr[:], scalar1=1e-10)
                nc.vector.reciprocal(out=r[:], in_=r[:])
                rr.append(r)
            qeS = sb.tile([8, LC], BF16, name="qeS", tag="qeS")
            nc.scalar.copy(out=qeS[:], in_=pqe[:])

            # ---- output: o[l,:] = r[l] * (qeT[:,l] . emb_c) ----
            for i in range(LT):
                o = sb.tile([P, H], BF16, name=f"o{i}", tag=f"o{i}")
                for half in range(2):
                    po = psO.tile([P, H // 2], F32, name="po", tag="po")
                    nc.tensor.matmul(
                        out=po[:],
                        lhsT=qeS[:, i * P : (i + 1) * P],
                        rhs=ec[:, half * (H // 2) : (half + 1) * (H // 2)],
                        start=True,
                        stop=True,
                    )
                    if (i + half) % 2 == 0:
                        nc.scalar.activation(
                            out=o[:, half * (H // 2) : (half + 1) * (H // 2)],
                            in_=po[:],
                            func=mybir.ActivationFunctionType.Copy,
                            bias=0.0, scale=rr[i][:, 0:1],
                        )
                    else:
                        nc.vector.tensor_scalar(
                            out=o[:, half * (H // 2) : (half + 1) * (H // 2)],
                            in0=po[:],
                            scalar1=rr[i][:, 0:1], scalar2=None,
                            op0=mybir.AluOpType.mult,
                        )
                nc.sync.dma_start(out=o_d[i * P : (i + 1) * P, :], in_=o[:])

    nc.compile()
    return nc


def _get_nc():
    if "nc" not in _CACHE:
        _CACHE["nc"] = _build()
    return _CACHE["nc"]


def _prep_shared(inputs):
    """Host-side packing shared across cores."""
    ea = np.asarray(inputs["emb_a"], dtype=np.float32)
    lw = np.asarray(inputs["lin_w"], dtype=np.float32)
    lb = np.asarray(inputs["lin_b"], dtype=np.float32)
    ec = np.asarray(inputs["emb_c"], dtype=np.float32)
    ws = np.asarray(inputs["word_seq"]).astype(np.int64)

    # 16 * lin_w.T extended with 16 * lin_b in the last column, fp8-packed
    lwx = np.zeros((H, EP), dtype=F8NP)
    lwx[:, :E] = (LWS * lw.T).astype(F8NP)
    lwx[:, EP - 1] = (LWS * lb).astype(F8NP)
    lwx_p = np.ascontiguousarray(
        lwx.reshape(HT, P, EP).transpose(1, 0, 2).reshape(P, HT * EP)
    )

    oh = np.zeros((P, OHC), dtype=BF)
    for n0 in range(NL - 1):
        oh[:, 8 * n0 + (n0 + 1)] = BF(1.0)  # label n0+1 -> pqe row n0+1
        oh[:, 8 * n0 + 0] = BF(1.0)         # normalizer -> pqe row 0
    oh[0, 40:48] = BF(1.0)                  # ones column for s-transpose

    ec8 = np.zeros((8, H), dtype=BF)
    ec8[1:NL] = ec[1:].astype(BF)  # row 0 is the normalizer slot -> zero

    # per-batch gathered rows (16x fp8), extended with the bias column,
    # transposed
    gT_pb = []
    for b in range(B):
        g_ext = np.zeros((W, EP), dtype=F8NP)
        g_ext[:, :E] = (GS * ea[ws[b]]).astype(F8NP)
        g_ext[:, EP - 1] = F8NP(GS)
        gT = np.ascontiguousarray(g_ext.T)  # [EP, W]
        gT_pb.append(
            np.ascontiguousarray(
                gT.reshape(ET, P, W).transpose(1, 0, 2).reshape(P, ET * W)
            )
        )
    return lwx_p, oh, ec8, gT_pb


def _core_map(inputs, lwx_p, oh, ec8, gT_pb, core):
    hs = np.asarray(inputs["hidden_state"], dtype=np.float32)
    lvm = np.asarray(inputs["label_value_matrix"]).astype(np.int8)
    b, half = divmod(core, 2)
    lsl = slice(half * LC, (half + 1) * LC)
    hT = hs[b, lsl].T.astype(F8NP)  # [H, LC]
    hT_p = np.ascontiguousarray(
        hT.reshape(HT, P, LC).transpose(1, 0, 2).reshape(P, HT * LC)
    )
    labT = lvm[b, lsl].T  # [W, LC] int8
    labT_p = np.ascontiguousarray(
        labT.reshape(WT, P, LC).transpose(1, 0, 2).reshape(P, WT * LC)
    )
    return {
        "hT": hT_p,
        "lwx": lwx_p,
        "gT": gT_pb[b],
        "label": labT_p,
        "emb_c": ec8,
        "oh": oh,
    }


def kernel(**inputs):
    nc = _get_nc()
    lwx_p, oh, ec8, gT_pb = _prep_shared(inputs)
    in_maps = [_core_map(inputs, lwx_p, oh, ec8, gT_pb, c) for c in range(NCORES)]

    res = bass_utils.run_bass_kernel_spmd(
        nc, in_maps, core_ids=list(range(NCORES)), trace=TRACE
    )
    _CACHE["last_result"] = res

    out = np.empty((B, L, H), np.float32)
    for c in range(NCORES):
        b, half = divmod(c, 2)
        out[b, half * LC : (half + 1) * LC] = np.asarray(
            res.results[c]["o"]
        ).astype(np.float32)
    return out
